# revision 3
# baseline (speedup 1.0000x reference)
"""BlockWiseHistogramEncoder Trainium2 kernel — radix-16 PE-matmul version.

Input  x: [16, 1, 512, 512] int32, values in [0, 64).
Output:   [16, 1024, 65] float32; out[b, l, 1+v] = count(v in block l)/256.

Sharding: pure data parallel over batch — 2 batches per core on 8 cores.

Per-core algorithm (per batch image):
  1. DMA HBM -> SBUF column-strip layout x_strip [32=bc, 8192=(row, c)],
     then 8 SBUF->SBUF remap DMAs -> block-major x_i32 [128=block, 2048].
  2. GPSIMD converts int32 -> bf16.
  3. PE transposes 16 [128,128] slabs -> x_T [128=elem-slot, 2048=(slab,block)]
     (via PSUM, evacuated by DVE/GPSIMD).
  4. DVE: lo = mod(x_T, 8); hi8 = x_T - lo; then 8+8 is_equal passes build
     one-hot indicator matrices ind_hi/ind_lo [128, (slab, dig, blk16)]
     (bf16, 4x DVE perf mode eligible).
  5. PE: per (tile, group-of-16-blocks): psum[m=(h,b), n=(l,b')] =
     sum_k hi_oh[k,(h,b)] * lo_oh[k,(l,b')] accumulated over the 2 elem
     chunks. Diagonal b==b' holds the 8x8 radix histogram of each block.
  6. Evac PSUM -> SBUF bf16 with *1/256 scale (split over ACT/DVE/GPSIMD),
     dump [128, 8192] bf16 per batch to HBM.
Host: extracts the block-diagonal entries (pure indexing) and emits fp32.
All arithmetic is exact (0/1 bf16 products, fp32 PSUM accum, k/256 is
exactly representable in bf16), so rel err vs the reference is 0.
"""
import sys

if "/opt/trn_rl_repo" not in sys.path:
    sys.path.insert(0, "/opt/trn_rl_repo")

import numpy as np

N_CORES = 8
B_PER_CORE = 2
H = W = 512
NC_CLS = 64
BLK = 16
HB = H // BLK            # 32 blocks per side
L = HB * HB              # 1024 blocks per image
E = BLK * BLK            # 256 elems per block
T_TILES = 8              # 128-block tiles per image
NSLAB = 16               # (tile, chunk) transposed slabs per image
SCOLS = 8192             # dump cols per image: 64 (t,g) tiles x 128

_nc_cache = None
_run_cache = None


def _build():
    import concourse.bacc as bacc
    import concourse.mybir as mybir
    import concourse.tile as tile
    from concourse.masks import make_identity

    nc = bacc.Bacc("TRN2", target_bir_lowering=False, debug=False)
    x = nc.dram_tensor("x_in", [B_PER_CORE, H, W], mybir.dt.int32,
                       kind="ExternalInput")
    y = nc.dram_tensor("y_dump", [B_PER_CORE, 128, SCOLS], mybir.dt.bfloat16,
                       kind="ExternalOutput")
    bf16 = mybir.dt.bfloat16

    with tile.TileContext(nc) as tc:
        with tc.tile_pool(name="cst", bufs=1) as c_pool, \
             tc.tile_pool(name="strip", bufs=1) as strip_pool, \
             tc.tile_pool(name="blk", bufs=2) as blk_pool, \
             tc.tile_pool(name="xt", bufs=2) as xt_pool, \
             tc.tile_pool(name="ind", bufs=1) as ind_pool, \
             tc.tile_pool(name="out", bufs=2) as out_pool, \
             tc.psum_pool(name="ptr", bufs=2) as ptr_pool, \
             tc.psum_pool(name="ph", bufs=4) as ph_pool:
            ident = c_pool.tile([128, 128], bf16)
            make_identity(nc, ident[:])

            for b in range(B_PER_CORE):
                # ---- 1. load + block-major remap ----
                x_strip = strip_pool.tile([HB, 16 * H], mybir.dt.int32,
                                          tag="strip")
                nc.sync.dma_start(
                    x_strip[:].rearrange("bc (r c) -> bc r c", c=BLK),
                    x.ap()[b].rearrange("r (bc c) -> bc r c", c=BLK))
                x_i32 = blk_pool.tile([128, 2048], mybir.dt.int32, tag="xi")
                xs4 = x_strip[:].rearrange("bc (t g rc) -> bc t g rc",
                                           g=4, rc=E)
                for g in range(4):
                    nc.sync.dma_start(
                        x_i32[32 * g:32 * (g + 1), :].rearrange(
                            "bc (t rc) -> bc t rc", rc=E),
                        xs4[:, :, g, :])
                # ---- 2. convert ----
                x_bf = blk_pool.tile([128, 2048], bf16, tag="xbf")
                nc.gpsimd.tensor_copy(x_bf[:], x_i32[:])
                # ---- 3. transpose to elem-slot-major ----
                x_T = xt_pool.tile([128, 2048], bf16, tag="xT")
                for half in range(2):
                    pt = ptr_pool.tile([128, 1024], bf16, tag="ptr")
                    for j in range(8):
                        s = half * 8 + j
                        nc.tensor.transpose(
                            pt[:, 128 * j:128 * (j + 1)],
                            x_bf[:, 128 * s:128 * (s + 1)], ident[:])
                    dst_xt = x_T[:, 1024 * half:1024 * (half + 1)]
                    if half == 0:
                        nc.vector.tensor_copy(dst_xt, pt[:])
                    else:
                        nc.scalar.activation(
                            dst_xt, pt[:],
                            mybir.ActivationFunctionType.Copy,
                            bias=0.0, scale=1.0)
                # ---- 4. digits + one-hot builds ----
                # lo = x mod 8 via 3 conditional-subtract rounds (is_ge masks
                # are the bits of hi); hi8 = x - lo. Only is_ge/mult/add ALUs.
                lo = xt_pool.tile([128, 2048], bf16, tag="lo")
                msk = xt_pool.tile([128, 2048], bf16, tag="msk")
                srcs = [x_T, lo, lo]
                for i, sub in enumerate((32.0, 16.0, 8.0)):
                    nc.vector.tensor_scalar(
                        msk[:], srcs[i][:], sub, 0.0,
                        mybir.AluOpType.is_ge, mybir.AluOpType.add)
                    nc.vector.scalar_tensor_tensor(
                        lo[:], msk[:], -sub, srcs[i][:],
                        mybir.AluOpType.mult, mybir.AluOpType.add)
                hi8 = xt_pool.tile([128, 2048], bf16, tag="hi8")
                nc.vector.scalar_tensor_tensor(
                    hi8[:], lo[:], -1.0, x_T[:],
                    mybir.AluOpType.mult, mybir.AluOpType.add)
                ind_hi = ind_pool.tile([128, 16384], bf16, tag="ihi")
                ind_lo = ind_pool.tile([128, 16384], bf16, tag="ilo")
                ihi4 = ind_hi[:].rearrange("p (s d e) -> p s d e", d=8, e=16)
                ilo4 = ind_lo[:].rearrange("p (s d e) -> p s d e", d=8, e=16)
                src3 = [t3[:].rearrange("p (s e) -> p s e", e=16)
                        for t3 in (hi8, lo)]
                for d in range(8):
                    eq_eng = nc.vector if d < 5 else nc.gpsimd
                    eq_eng.tensor_scalar(
                        ihi4[:, :, d, :], src3[0], float(8 * d), 0.0,
                        mybir.AluOpType.is_equal, mybir.AluOpType.add)
                    eq_eng.tensor_scalar(
                        ilo4[:, :, d, :], src3[1], float(d), 0.0,
                        mybir.AluOpType.is_equal, mybir.AluOpType.add)
                # ---- 5./6. matmuls + evac ----
                S = out_pool.tile([128, SCOLS], bf16, tag="S")
                for t in range(T_TILES):
                    for k in range(2):
                        ph = ph_pool.tile([128, 512], mybir.dt.float32,
                                          tag="ph")
                        for j in range(4):
                            g = 4 * k + j
                            for c2 in range(2):
                                off = 128 * ((2 * t + c2) * 8 + g)
                                nc.tensor.matmul(
                                    ph[:, 128 * j:128 * (j + 1)],
                                    ind_hi[:, off:off + 128],
                                    ind_lo[:, off:off + 128],
                                    start=(c2 == 0), stop=(c2 == 1))
                        dst = S[:, 1024 * t + 512 * k:1024 * t + 512 * (k + 1)]
                        nc.scalar.activation(
                            dst, ph[:],
                            mybir.ActivationFunctionType.Copy,
                            bias=0.0, scale=1.0 / E)
                nc.sync.dma_start(y.ap()[b], S[:])
    nc.compile()
    return nc


def _get_nc():
    global _nc_cache
    if _nc_cache is None:
        _nc_cache = _build()
    return _nc_cache


def _get_runner():
    """Build the sharded jitted executable once."""
    global _run_cache
    if _run_cache is not None:
        return _run_cache

    import jax
    from jax.sharding import Mesh, PartitionSpec
    from jax.experimental.shard_map import shard_map
    import concourse.mybir as mybir
    from concourse.bass2jax import (
        _bass_exec_p, install_neuronx_cc_hook, partition_id_tensor)

    nc = _get_nc()
    install_neuronx_cc_hook()

    partition_name = (nc.partition_id_tensor.name
                      if nc.partition_id_tensor else None)
    in_names, out_names, out_avals = [], [], []
    for alloc in nc.m.functions[0].allocations:
        if not isinstance(alloc, mybir.MemoryLocationSet):
            continue
        name = alloc.memorylocations[0].name
        if alloc.kind == "ExternalInput":
            if name != partition_name:
                in_names.append(name)
        elif alloc.kind == "ExternalOutput":
            out_names.append(name)
            out_avals.append(jax.core.ShapedArray(
                tuple(alloc.tensor_shape), mybir.dt.np(alloc.dtype)))
    n_params = len(in_names)
    n_outs = len(out_avals)
    all_in_names = list(in_names) + list(out_names)
    if partition_name is not None:
        all_in_names.append(partition_name)

    def _body(*args):
        operands = list(args)
        if partition_name is not None:
            operands.append(partition_id_tensor())
        outs = _bass_exec_p.bind(
            *operands,
            out_avals=tuple(out_avals),
            in_names=tuple(all_in_names),
            out_names=tuple(out_names),
            lowering_input_output_aliases=(),
            sim_require_finite=True,
            sim_require_nnan=True,
            nc=nc,
        )
        return tuple(outs)

    devices = jax.devices()[:N_CORES]
    mesh = Mesh(np.asarray(devices), ("core",))
    in_specs = (PartitionSpec("core"),) * (n_params + n_outs)
    out_specs = (PartitionSpec("core"),) * n_outs
    donate = tuple(range(n_params, n_params + n_outs))
    sharded = jax.jit(
        shard_map(_body, mesh=mesh, in_specs=in_specs, out_specs=out_specs,
                  check_rep=False),
        donate_argnums=donate, keep_unused=True)

    zero_shapes = [(N_CORES * a.shape[0], *a.shape[1:]) for a in out_avals]
    zero_dtypes = [a.dtype for a in out_avals]

    def run(concat_inputs):
        zeros = [np.zeros(s, d) for s, d in zip(zero_shapes, zero_dtypes)]
        out_arrs = sharded(*concat_inputs, *zeros)
        return {name: np.asarray(out_arrs[i]) for i, name in
                enumerate(out_names)}

    _run_cache = run
    return run


def _extract(dump: np.ndarray) -> np.ndarray:
    """dump [16, 128, 8192] bf16 -> y [16, 1024, 65] fp32 (pure indexing)."""
    d = dump.astype(np.float32)
    # partition = (h:8, b:16); cols = (t:8, g:8, l:8, b':16)
    d7 = d.reshape(16, 8, 16, 8, 8, 8, 16)
    diag = np.diagonal(d7, axis1=2, axis2=6)     # [16, h, t, g, l, b]
    y = np.zeros((16, L, NC_CLS + 1), np.float32)
    # block m = 128 t + 16 g + b ; class col = 1 + 8h + l
    y[:, :, 1:] = diag.transpose(0, 2, 3, 5, 1, 4).reshape(16, L, 64)
    return y


def kernel(x: np.ndarray) -> np.ndarray:
    assert x.shape == (16, 1, H, W) and x.dtype == np.int32, (x.shape, x.dtype)
    run = _get_runner()
    xs = np.ascontiguousarray(x[:, 0])          # [16, 512, 512] = concat of
    out = run([xs])["y_dump"]                   # 8 cores' [2, 512, 512]
    return _extract(out.reshape(16, 128, SCOLS))


# revision 4
# speedup vs baseline: 5.9508x; 5.9508x over previous
"""BlockWiseHistogramEncoder Trainium2 kernel — radix-16 PE-matmul version.

Input  x: [16, 1, 512, 512] int32, values in [0, 64).
Output:   [16, 1024, 65] float32; out[b, l, 1+v] = count(v in block l)/256.

Sharding: pure data parallel over batch — 2 batches per core on 8 cores.

Per-core algorithm (per batch image):
  1. DMA HBM -> SBUF column-strip layout x_strip [32=bc, 8192=(row, c)],
     then 8 SBUF->SBUF remap DMAs -> block-major x_i32 [128=block, 2048].
  2. GPSIMD converts int32 -> bf16.
  3. PE transposes 16 [128,128] slabs -> x_T [128=elem-slot, 2048=(slab,block)]
     (via PSUM, evacuated by DVE/GPSIMD).
  4. DVE: lo = mod(x_T, 8); hi8 = x_T - lo; then 8+8 is_equal passes build
     one-hot indicator matrices ind_hi/ind_lo [128, (slab, dig, blk16)]
     (bf16, 4x DVE perf mode eligible).
  5. PE: per (tile, group-of-16-blocks): psum[m=(h,b), n=(l,b')] =
     sum_k hi_oh[k,(h,b)] * lo_oh[k,(l,b')] accumulated over the 2 elem
     chunks. Diagonal b==b' holds the 8x8 radix histogram of each block.
  6. Evac PSUM -> SBUF bf16 with *1/256 scale (split over ACT/DVE/GPSIMD),
     dump [128, 8192] bf16 per batch to HBM.
Host: extracts the block-diagonal entries (pure indexing) and emits fp32.
All arithmetic is exact (0/1 bf16 products, fp32 PSUM accum, k/256 is
exactly representable in bf16), so rel err vs the reference is 0.
"""
import sys

if "/opt/trn_rl_repo" not in sys.path:
    sys.path.insert(0, "/opt/trn_rl_repo")

import numpy as np

N_CORES = 8
B_PER_CORE = 2
H = W = 512
NC_CLS = 64
BLK = 16
HB = H // BLK            # 32 blocks per side
L = HB * HB              # 1024 blocks per image
E = BLK * BLK            # 256 elems per block
T_TILES = 8              # 128-block tiles per image
NSLAB = 16               # (tile, chunk) transposed slabs per image
SCOLS = 8192             # dump cols per image: 64 (t,g) tiles x 128

_nc_cache = None
_run_cache = None


def _build():
    import concourse.bacc as bacc
    import concourse.mybir as mybir
    import concourse.tile as tile
    from concourse.masks import make_identity

    nc = bacc.Bacc("TRN2", target_bir_lowering=False, debug=False)
    x = nc.dram_tensor("x_in", [B_PER_CORE, H, W], mybir.dt.int32,
                       kind="ExternalInput")
    y = nc.dram_tensor("y_dump", [B_PER_CORE, 128, SCOLS], mybir.dt.bfloat16,
                       kind="ExternalOutput")
    bf16 = mybir.dt.bfloat16

    with tile.TileContext(nc) as tc:
        with tc.tile_pool(name="cst", bufs=1) as c_pool, \
             tc.tile_pool(name="strip", bufs=1) as strip_pool, \
             tc.tile_pool(name="blk", bufs=2) as blk_pool, \
             tc.tile_pool(name="xt", bufs=2) as xt_pool, \
             tc.tile_pool(name="ind", bufs=1) as ind_pool, \
             tc.tile_pool(name="out", bufs=2) as out_pool, \
             tc.psum_pool(name="ptr", bufs=2) as ptr_pool, \
             tc.psum_pool(name="ph", bufs=4) as ph_pool:
            ident = c_pool.tile([128, 128], bf16)
            make_identity(nc, ident[:])

            for b in range(B_PER_CORE):
                # ---- 1. load + block-major remap ----
                x_strip = strip_pool.tile([HB, 16 * H], mybir.dt.int32,
                                          tag="strip")
                nc.sync.dma_start(
                    x_strip[:].rearrange("bc (r c) -> bc r c", c=BLK),
                    x.ap()[b].rearrange("r (bc c) -> bc r c", c=BLK))
                x_i32 = blk_pool.tile([128, 2048], mybir.dt.int32, tag="xi")
                xs4 = x_strip[:].rearrange("bc (t g rc) -> bc t g rc",
                                           g=4, rc=E)
                for g in range(4):
                    nc.sync.dma_start(
                        x_i32[32 * g:32 * (g + 1), :].rearrange(
                            "bc (t rc) -> bc t rc", rc=E),
                        xs4[:, :, g, :])
                # ---- 2. convert ----
                x_bf = blk_pool.tile([128, 2048], bf16, tag="xbf")
                nc.gpsimd.tensor_copy(x_bf[:], x_i32[:])
                # ---- 3. transpose to elem-slot-major ----
                x_T = xt_pool.tile([128, 2048], bf16, tag="xT")
                for half in range(2):
                    pt = ptr_pool.tile([128, 1024], bf16, tag="ptr")
                    for j in range(8):
                        s = half * 8 + j
                        nc.tensor.transpose(
                            pt[:, 128 * j:128 * (j + 1)],
                            x_bf[:, 128 * s:128 * (s + 1)], ident[:])
                    dst_xt = x_T[:, 1024 * half:1024 * (half + 1)]
                    if half == 0:
                        nc.vector.tensor_copy(dst_xt, pt[:])
                    else:
                        nc.scalar.activation(
                            dst_xt, pt[:],
                            mybir.ActivationFunctionType.Copy,
                            bias=0.0, scale=1.0)
                # ---- 4. digits + one-hot builds ----
                # lo = x mod 8 via 3 conditional-subtract rounds (is_ge masks
                # are the bits of hi); hi8 = x - lo. Only is_ge/mult/add ALUs.
                lo = xt_pool.tile([128, 2048], bf16, tag="lo")
                msk = xt_pool.tile([128, 2048], bf16, tag="msk")
                srcs = [x_T, lo, lo]
                for i, sub in enumerate((32.0, 16.0, 8.0)):
                    nc.vector.tensor_scalar(
                        msk[:], srcs[i][:], sub, 0.0,
                        mybir.AluOpType.is_ge, mybir.AluOpType.add)
                    nc.vector.scalar_tensor_tensor(
                        lo[:], msk[:], -sub, srcs[i][:],
                        mybir.AluOpType.mult, mybir.AluOpType.add)
                hi8 = xt_pool.tile([128, 2048], bf16, tag="hi8")
                nc.vector.scalar_tensor_tensor(
                    hi8[:], lo[:], -1.0, x_T[:],
                    mybir.AluOpType.mult, mybir.AluOpType.add)
                ind_hi = ind_pool.tile([128, 16384], bf16, tag="ihi")
                ind_lo = ind_pool.tile([128, 16384], bf16, tag="ilo")
                ihi4 = ind_hi[:].rearrange("p (s d e) -> p s d e", d=8, e=16)
                ilo4 = ind_lo[:].rearrange("p (s d e) -> p s d e", d=8, e=16)
                src3 = [t3[:].rearrange("p (s e) -> p s e", e=16)
                        for t3 in (hi8, lo)]
                for d in range(8):
                    nc.vector.tensor_scalar(
                        ihi4[:, :, d, :], src3[0], float(8 * d), 0.0,
                        mybir.AluOpType.is_equal, mybir.AluOpType.add)
                    nc.vector.tensor_scalar(
                        ilo4[:, :, d, :], src3[1], float(d), 0.0,
                        mybir.AluOpType.is_equal, mybir.AluOpType.add)
                # ---- 5./6. matmuls + evac ----
                S = out_pool.tile([128, SCOLS], bf16, tag="S")
                for t in range(T_TILES):
                    for k in range(2):
                        ph = ph_pool.tile([128, 512], mybir.dt.float32,
                                          tag="ph")
                        for j in range(4):
                            g = 4 * k + j
                            for c2 in range(2):
                                off = 128 * ((2 * t + c2) * 8 + g)
                                nc.tensor.matmul(
                                    ph[:, 128 * j:128 * (j + 1)],
                                    ind_hi[:, off:off + 128],
                                    ind_lo[:, off:off + 128],
                                    start=(c2 == 0), stop=(c2 == 1))
                        dst = S[:, 1024 * t + 512 * k:1024 * t + 512 * (k + 1)]
                        if (t + k) % 4 == 0:
                            nc.vector.tensor_scalar(
                                dst, ph[:], 1.0 / E, 0.0,
                                mybir.AluOpType.mult,
                                mybir.AluOpType.add)
                        else:
                            nc.scalar.activation(
                                dst, ph[:],
                                mybir.ActivationFunctionType.Copy,
                                bias=0.0, scale=1.0 / E)
                nc.sync.dma_start(y.ap()[b], S[:])
    nc.compile()
    return nc


def _get_nc():
    global _nc_cache
    if _nc_cache is None:
        _nc_cache = _build()
    return _nc_cache


def _get_runner():
    """Build the sharded jitted executable once."""
    global _run_cache
    if _run_cache is not None:
        return _run_cache

    import jax
    from jax.sharding import Mesh, PartitionSpec
    from jax.experimental.shard_map import shard_map
    import concourse.mybir as mybir
    from concourse.bass2jax import (
        _bass_exec_p, install_neuronx_cc_hook, partition_id_tensor)

    nc = _get_nc()
    install_neuronx_cc_hook()

    partition_name = (nc.partition_id_tensor.name
                      if nc.partition_id_tensor else None)
    in_names, out_names, out_avals = [], [], []
    for alloc in nc.m.functions[0].allocations:
        if not isinstance(alloc, mybir.MemoryLocationSet):
            continue
        name = alloc.memorylocations[0].name
        if alloc.kind == "ExternalInput":
            if name != partition_name:
                in_names.append(name)
        elif alloc.kind == "ExternalOutput":
            out_names.append(name)
            out_avals.append(jax.core.ShapedArray(
                tuple(alloc.tensor_shape), mybir.dt.np(alloc.dtype)))
    n_params = len(in_names)
    n_outs = len(out_avals)
    all_in_names = list(in_names) + list(out_names)
    if partition_name is not None:
        all_in_names.append(partition_name)

    def _body(*args):
        operands = list(args)
        if partition_name is not None:
            operands.append(partition_id_tensor())
        outs = _bass_exec_p.bind(
            *operands,
            out_avals=tuple(out_avals),
            in_names=tuple(all_in_names),
            out_names=tuple(out_names),
            lowering_input_output_aliases=(),
            sim_require_finite=True,
            sim_require_nnan=True,
            nc=nc,
        )
        return tuple(outs)

    devices = jax.devices()[:N_CORES]
    mesh = Mesh(np.asarray(devices), ("core",))
    in_specs = (PartitionSpec("core"),) * (n_params + n_outs)
    out_specs = (PartitionSpec("core"),) * n_outs
    donate = tuple(range(n_params, n_params + n_outs))
    sharded = jax.jit(
        shard_map(_body, mesh=mesh, in_specs=in_specs, out_specs=out_specs,
                  check_rep=False),
        donate_argnums=donate, keep_unused=True)

    zero_shapes = [(N_CORES * a.shape[0], *a.shape[1:]) for a in out_avals]
    zero_dtypes = [a.dtype for a in out_avals]

    def run(concat_inputs):
        zeros = [np.zeros(s, d) for s, d in zip(zero_shapes, zero_dtypes)]
        out_arrs = sharded(*concat_inputs, *zeros)
        return {name: np.asarray(out_arrs[i]) for i, name in
                enumerate(out_names)}

    _run_cache = run
    return run


def _extract(dump: np.ndarray) -> np.ndarray:
    """dump [16, 128, 8192] bf16 -> y [16, 1024, 65] fp32 (pure indexing)."""
    d = dump.astype(np.float32)
    # partition = (h:8, b:16); cols = (t:8, g:8, l:8, b':16)
    d7 = d.reshape(16, 8, 16, 8, 8, 8, 16)
    diag = np.diagonal(d7, axis1=2, axis2=6)     # [16, h, t, g, l, b]
    y = np.zeros((16, L, NC_CLS + 1), np.float32)
    # block m = 128 t + 16 g + b ; class col = 1 + 8h + l
    y[:, :, 1:] = diag.transpose(0, 2, 3, 5, 1, 4).reshape(16, L, 64)
    return y


def kernel(x: np.ndarray) -> np.ndarray:
    assert x.shape == (16, 1, H, W) and x.dtype == np.int32, (x.shape, x.dtype)
    run = _get_runner()
    xs = np.ascontiguousarray(x[:, 0])          # [16, 512, 512] = concat of
    out = run([xs])["y_dump"]                   # 8 cores' [2, 512, 512]
    return _extract(out.reshape(16, 128, SCOLS))
